# revision 7
# baseline (speedup 1.0000x reference)
"""Trainium2 Bass kernel: standard multi-head attention (B=2, S=2048, H=16, D=128, fp32).

Sharding: head-parallel across 8 NeuronCores (2 heads per core, both batches),
zero cross-core communication.

Host side (part of sharding): per core, Q and K head-slices are transposed to
[d, s] and cast to fp16; V is cast to fp16 and augmented with a ones column
(so the PV matmul accumulates the softmax denominator for free); the key
padding mask becomes an additive bias laid out per k-tile.

Per-core device program, per (batch, head) unit:
  - DMA qT, kT, V_aug (fp16) straight into SBUF.
  - For each 1024-wide q-chunk:
      Phase A (16 k-tiles): S^T[k, q] = (K Q^T) via PE matmul (fp16 in, fp32 PSUM),
        P^T = exp(scale * S^T + mask_bias[k]) on ACT -> 16 fp16 tiles in SBUF.
      Phase B (8 q-tiles, sequential PSUM accumulation groups, one bank each):
        out[q, d'] += P^T-slice^T @ V_aug over 16 k-tiles; d'=128 is the
        denominator column.
      Normalize out[:, :128] by 1/out[:, 128] (DVE), DMA to HBM.
  Phase B of chunk c is software-pipelined against Phase A of chunk c+1.

All accumulation fp32; matmul inputs fp16 (10-bit mantissa, full PE rate).
"""

import numpy as np

B, S, H, D = 2, 2048, 16, 128
NCORES = 8
H2 = H // NCORES          # heads per core
KTILES = S // 128         # 16
VW = D + 2                # V_aug row width: 128 d + ones col + pad
QCHUNK = 1024
NQC = S // QCHUNK         # 2
QT = QCHUNK // 128        # 8 q-tiles per chunk
SCALE = 1.0 / float(np.sqrt(D))
MASK_NEG = -30.0

_CACHE = {}


def _build_program():
    from contextlib import ExitStack

    import concourse.tile as tile
    from concourse import bacc, mybir

    f32 = mybir.dt.float32
    f16 = mybir.dt.float16

    nc = bacc.Bacc("TRN2", target_bir_lowering=False, debug=False, num_devices=NCORES)
    qt_d = nc.dram_tensor("qt", [B, H2, D, S], f16, kind="ExternalInput").ap()
    kt_d = nc.dram_tensor("kt", [B, H2, D, S], f16, kind="ExternalInput").ap()
    va_d = nc.dram_tensor("va", [B, H2, 128, KTILES, VW], f16, kind="ExternalInput").ap()
    bias_d = nc.dram_tensor("bias", [B, 128, KTILES], f32, kind="ExternalInput").ap()
    o_d = nc.dram_tensor("o", [B, S, H2, D], f32, kind="ExternalOutput").ap()

    EXP = mybir.ActivationFunctionType.Exp

    with tile.TileContext(nc) as tc, ExitStack() as ctx:
        tpool = ctx.enter_context(tc.tile_pool(name="tpool", bufs=2))
        vpool = ctx.enter_context(tc.tile_pool(name="vpool", bufs=2))
        bpool = ctx.enter_context(tc.tile_pool(name="bpool", bufs=1))
        ppool = ctx.enter_context(tc.tile_pool(name="ppool", bufs=34))
        opool = ctx.enter_context(tc.tile_pool(name="opool", bufs=2))
        rpool = ctx.enter_context(tc.tile_pool(name="rpool", bufs=4))
        st_ps = ctx.enter_context(tc.tile_pool(name="st_ps", bufs=2, space="PSUM"))
        o_ps = ctx.enter_context(tc.tile_pool(name="o_ps", bufs=4, space="PSUM"))

        # warm up the ACT exp table before any data arrives (table load ~2.7us)
        warm = rpool.tile([128, 1], f32, name="warm", tag="warm")
        nc.vector.memset(warm[:], 0.0)
        nc.scalar.activation(warm[:], warm[:], EXP, bias=0.0, scale=1.0)

        # mask bias for both batches: bias_d is [B, 128, KTILES] (partition-major)
        bias_sb = bpool.tile([128, B, KTILES], f32, name="bias_sb", tag="bias")
        nc.sync.dma_start(bias_sb[:], bias_d.rearrange("b p t -> p b t"))

        units = [(b, h) for b in range(B) for h in range(H2)]
        chunks = [(u, qc) for u in range(len(units)) for qc in range(NQC)]

        def prep(u):
            """DMA the unit's pre-transposed fp16 tensors into SBUF."""
            b, h = units[u]
            qt = tpool.tile([128, S], f16, name="qt_sb", tag="qt")
            nc.sync.dma_start(qt[:], qt_d[b, h])
            kt = tpool.tile([128, S], f16, name="kt_sb", tag="kt")
            nc.sync.dma_start(kt[:], kt_d[b, h])
            va = vpool.tile([128, KTILES, VW], f16, name="va_sb", tag="va")
            nc.sync.dma_start(va[:], va_d[b, h])
            return {"q": qt, "k": kt, "v": va}

        unit_tiles = {0: prep(0)}
        state = {}

        def emit_s(c, j):
            u, qc = chunks[c]
            b, h = units[u]
            tl = unit_tiles[u]
            q0 = qc * QCHUNK
            st = st_ps.tile([128, QCHUNK], f32, name="st", tag="st")
            for half in range(QCHUNK // 512):
                nc.tensor.matmul(
                    st[:, half * 512 : (half + 1) * 512],
                    lhsT=tl["k"][:, j * 128 : (j + 1) * 128],
                    rhs=tl["q"][:, q0 + half * 512 : q0 + (half + 1) * 512],
                    start=True,
                    stop=True,
                )
            pt = ppool.tile([128, QCHUNK], f16, name="pt", tag="pt")
            nc.scalar.activation(
                pt[:], st[:], EXP, bias=bias_sb[:, b, j : j + 1], scale=SCALE
            )
            state[c]["pt"].append(pt)

        # PV slot order: two halves of 4 concurrent q-tile groups, j-major
        # inside each half, so groups finish as soon as their last P^T tile
        # exists (short pipeline tail on the final chunk).
        PV_ORDER = [
            (half * 4 + t4, j)
            for half in range(QT // 8 * 2)
            for j in range(KTILES)
            for t4 in range(4)
        ]

        def emit_pv_steps(c, step):
            """Emit 8 PV matmuls for chunk c, plus drains as groups close."""
            stt = state[c]
            for t, j in PV_ORDER[step * 8 : step * 8 + 8]:
                if j == 0:
                    stt["oacc"][t] = o_ps.tile(
                        [128, D + 1], f32, name="oacc", tag="oacc"
                    )
                nc.tensor.matmul(
                    stt["oacc"][t][:],
                    lhsT=stt["pt"][j][:, t * 128 : (t + 1) * 128],
                    rhs=stt["v16"][:, j, 0 : D + 1],
                    start=(j == 0),
                    stop=(j == KTILES - 1),
                )
                if j == KTILES - 1:
                    rec = rpool.tile([128, 1], f32, name="rec", tag="rec")
                    nc.vector.reciprocal(rec[:], stt["oacc"][t][:, D : D + 1])
                    nc.vector.tensor_scalar_mul(
                        stt["osb"][:, t, :], stt["oacc"][t][:, 0:D], rec[:]
                    )

        def finish_chunk(c):
            u, qc = chunks[c]
            b, h = units[u]
            stt = state[c]
            nc.sync.dma_start(
                o_d[b, :, h, :].rearrange("(cc t p) d -> cc p t d", cc=NQC, p=128)[qc],
                stt["osb"][:],
            )
            del state[c]

        nchunks = len(chunks)
        for c in range(nchunks + 1):
            if c < nchunks:
                u, qc = chunks[c]
                state[c] = {
                    "pt": [],
                    "oacc": {},
                    "v16": unit_tiles[u]["v"],
                    "osb": opool.tile([128, QT, D], f32, name="osb", tag="osb"),
                }
                # prefetch next unit's tensors one chunk ahead
                if qc == NQC - 1 and u + 1 < len(units):
                    unit_tiles[u + 1] = prep(u + 1)
            for step in range(KTILES):
                if c < nchunks:
                    emit_s(c, step)
                if c > 0:
                    emit_pv_steps(c - 1, step)
            if c > 0:
                finish_chunk(c - 1)

    nc.compile()
    return nc


def _get_program():
    if "nc" not in _CACHE:
        _CACHE["nc"] = _build_program()
    return _CACHE["nc"]


def make_core_inputs(q, k, v, key_padding_mask):
    """Shard full inputs into per-core input maps (host side).

    Layout work done here (part of sharding): head-slice, transpose Q/K to
    [d, s], cast to fp16, build ones-augmented V, mask -> additive bias.
    """
    q = np.asarray(q, dtype=np.float32)
    k = np.asarray(k, dtype=np.float32)
    v = np.asarray(v, dtype=np.float32)

    bias = np.where(key_padding_mask, 0.0, MASK_NEG).astype(np.float32)
    # bias[b, s] with s = 128*t + p  ->  [B, 128(p), KTILES(t)]
    bias = np.ascontiguousarray(bias.reshape(B, KTILES, 128).transpose(0, 2, 1))

    # [B, S, H, D] -> [B, H, D, S] fp16
    qt = np.ascontiguousarray(q.transpose(0, 2, 3, 1).astype(np.float16))
    kt = np.ascontiguousarray(k.transpose(0, 2, 3, 1).astype(np.float16))
    # V_aug: [B, H, 128(p), KTILES(t), VW] fp16 with ones in column D
    va = np.zeros((B, H, 128, KTILES, VW), dtype=np.float16)
    # v[b, s, h, d] with s = 128*t + p
    va[:, :, :, :, 0:D] = (
        v.reshape(B, KTILES, 128, H, D).transpose(0, 3, 2, 1, 4).astype(np.float16)
    )
    va[:, :, :, :, D] = 1.0

    in_maps = []
    for c in range(NCORES):
        sl = slice(c * H2, (c + 1) * H2)
        in_maps.append(
            {
                "qt": np.ascontiguousarray(qt[:, sl]),
                "kt": np.ascontiguousarray(kt[:, sl]),
                "va": np.ascontiguousarray(va[:, sl]),
                "bias": bias,
            }
        )
    return in_maps


def assemble_output(results):
    """Concatenate per-core [B, S, H2, D] outputs along the head axis."""
    return np.concatenate([results[c]["o"] for c in range(NCORES)], axis=2)


def kernel(q, k, v, key_padding_mask):
    from concourse.bass_utils import run_bass_kernel_spmd

    nc = _get_program()
    in_maps = make_core_inputs(q, k, v, key_padding_mask)
    res = run_bass_kernel_spmd(nc, in_maps, list(range(NCORES)))
    return assemble_output(res.results)


# revision 9
# speedup vs baseline: 1.0403x; 1.0403x over previous
"""Trainium2 Bass kernel: standard multi-head attention (B=2, S=2048, H=16, D=128, fp32).

Sharding: head-parallel across 8 NeuronCores (2 heads per core, both batches),
zero cross-core communication.

Host side (part of sharding): per core, Q and K head-slices are transposed to
[d, s] and cast to fp16; V is cast to fp16 and augmented with a ones column
(so the PV matmul accumulates the softmax denominator for free); the key
padding mask becomes an additive bias laid out per k-tile.

Per-core device program, per (batch, head) unit:
  - DMA qT, kT, V_aug (fp16) straight into SBUF.
  - For each 1024-wide q-chunk:
      Phase A (16 k-tiles): S^T[k, q] = (K Q^T) via PE matmul (fp16 in, fp32 PSUM),
        P^T = exp(scale * S^T + mask_bias[k]) on ACT -> 16 fp16 tiles in SBUF.
      Phase B (8 q-tiles, sequential PSUM accumulation groups, one bank each):
        out[q, d'] += P^T-slice^T @ V_aug over 16 k-tiles; d'=128 is the
        denominator column.
      Normalize out[:, :128] by 1/out[:, 128] (DVE), DMA to HBM.
  Phase B of chunk c is software-pipelined against Phase A of chunk c+1.

All accumulation fp32; matmul inputs fp16 (10-bit mantissa, full PE rate).
"""

import numpy as np

B, S, H, D = 2, 2048, 16, 128
NCORES = 8
H2 = H // NCORES          # heads per core
KTILES = S // 128         # 16
VW = D + 2                # V_aug row width: 128 d + ones col + pad
QCHUNK = 1024
NQC = S // QCHUNK         # 2
QT = QCHUNK // 128        # 8 q-tiles per chunk
SCALE = 1.0 / float(np.sqrt(D))
MASK_NEG = -30.0

_CACHE = {}


def _build_program():
    from contextlib import ExitStack

    import concourse.tile as tile
    from concourse import bacc, mybir

    f32 = mybir.dt.float32
    f16 = mybir.dt.float16

    nc = bacc.Bacc("TRN2", target_bir_lowering=False, debug=False, num_devices=NCORES)
    qt_d = nc.dram_tensor("qt", [B, H2, D, S], f16, kind="ExternalInput").ap()
    kt_d = nc.dram_tensor("kt", [B, H2, D, S], f16, kind="ExternalInput").ap()
    va_d = nc.dram_tensor("va", [B, H2, 128, KTILES, VW], f16, kind="ExternalInput").ap()
    bias_d = nc.dram_tensor("bias", [B, 128, KTILES], f32, kind="ExternalInput").ap()
    o_d = nc.dram_tensor("o", [B, S, H2, D], f32, kind="ExternalOutput").ap()

    EXP = mybir.ActivationFunctionType.Exp

    with tile.TileContext(nc) as tc, ExitStack() as ctx:
        tpool = ctx.enter_context(tc.tile_pool(name="tpool", bufs=2))
        vpool = ctx.enter_context(tc.tile_pool(name="vpool", bufs=2))
        bpool = ctx.enter_context(tc.tile_pool(name="bpool", bufs=1))
        ppool = ctx.enter_context(tc.tile_pool(name="ppool", bufs=34))
        opool = ctx.enter_context(tc.tile_pool(name="opool", bufs=2))
        rpool = ctx.enter_context(tc.tile_pool(name="rpool", bufs=4))
        st_ps = ctx.enter_context(tc.tile_pool(name="st_ps", bufs=3, space="PSUM"))
        o_ps = ctx.enter_context(tc.tile_pool(name="o_ps", bufs=2, space="PSUM"))

        # warm up the ACT exp table before any data arrives (table load ~2.7us)
        warm = rpool.tile([128, 1], f32, name="warm", tag="warm")
        nc.vector.memset(warm[:], 0.0)
        nc.scalar.activation(warm[:], warm[:], EXP, bias=0.0, scale=1.0)

        # mask bias for both batches: bias_d is [B, 128, KTILES] (partition-major)
        bias_sb = bpool.tile([128, B, KTILES], f32, name="bias_sb", tag="bias")
        nc.sync.dma_start(bias_sb[:], bias_d.rearrange("b p t -> p b t"))

        units = [(b, h) for b in range(B) for h in range(H2)]
        chunks = [(u, qc) for u in range(len(units)) for qc in range(NQC)]

        def prep(u):
            """DMA the unit's pre-transposed fp16 tensors into SBUF."""
            b, h = units[u]
            qt = tpool.tile([128, S], f16, name="qt_sb", tag="qt")
            nc.sync.dma_start(qt[:], qt_d[b, h])
            kt = tpool.tile([128, S], f16, name="kt_sb", tag="kt")
            nc.sync.dma_start(kt[:], kt_d[b, h])
            va = vpool.tile([128, KTILES, VW], f16, name="va_sb", tag="va")
            nc.sync.dma_start(va[:], va_d[b, h])
            return {"q": qt, "k": kt, "v": va}

        unit_tiles = {0: prep(0)}
        state = {}

        def emit_s(c, j):
            u, qc = chunks[c]
            b, h = units[u]
            tl = unit_tiles[u]
            q0 = qc * QCHUNK
            st = st_ps.tile([128, QCHUNK], f32, name="st", tag="st")
            for half in range(QCHUNK // 512):
                nc.tensor.matmul(
                    st[:, half * 512 : (half + 1) * 512],
                    lhsT=tl["k"][:, j * 128 : (j + 1) * 128],
                    rhs=tl["q"][:, q0 + half * 512 : q0 + (half + 1) * 512],
                    start=True,
                    stop=True,
                )
            pt = ppool.tile([128, QCHUNK], f16, name="pt", tag="pt")
            nc.scalar.activation(
                pt[:], st[:], EXP, bias=bias_sb[:, b, j : j + 1], scale=SCALE
            )
            state[c]["pt"].append(pt)

        # PV slot order: q-tile-major — one accumulation group at a time,
        # each in its own PSUM bank (zero-region safety), double-buffered.
        PV_ORDER = [(t, j) for t in range(QT) for j in range(KTILES)]

        def emit_pv_steps(c, step):
            """Emit 8 PV matmuls for chunk c, plus drains as groups close."""
            stt = state[c]
            for t, j in PV_ORDER[step * 8 : step * 8 + 8]:
                if j == 0:
                    stt["oacc"][t] = o_ps.tile(
                        [128, D + 1], f32, name="oacc", tag="oacc"
                    )
                nc.tensor.matmul(
                    stt["oacc"][t][:],
                    lhsT=stt["pt"][j][:, t * 128 : (t + 1) * 128],
                    rhs=stt["v16"][:, j, 0 : D + 1],
                    start=(j == 0),
                    stop=(j == KTILES - 1),
                )
                if j == KTILES - 1:
                    rec = rpool.tile([128, 1], f32, name="rec", tag="rec")
                    nc.vector.reciprocal(rec[:], stt["oacc"][t][:, D : D + 1])
                    nc.vector.tensor_scalar_mul(
                        stt["osb"][:, t, :], stt["oacc"][t][:, 0:D], rec[:]
                    )

        def finish_chunk(c):
            u, qc = chunks[c]
            b, h = units[u]
            stt = state[c]
            nc.sync.dma_start(
                o_d[b, :, h, :].rearrange("(cc t p) d -> cc p t d", cc=NQC, p=128)[qc],
                stt["osb"][:],
            )
            del state[c]

        nchunks = len(chunks)
        for c in range(nchunks + 1):
            if c < nchunks:
                u, qc = chunks[c]
                state[c] = {
                    "pt": [],
                    "oacc": {},
                    "v16": unit_tiles[u]["v"],
                    "osb": opool.tile([128, QT, D], f32, name="osb", tag="osb"),
                }
                # prefetch next unit's tensors one chunk ahead
                if qc == NQC - 1 and u + 1 < len(units):
                    unit_tiles[u + 1] = prep(u + 1)
            for step in range(KTILES):
                if c < nchunks:
                    emit_s(c, step)
                if c > 0:
                    emit_pv_steps(c - 1, step)
            if c > 0:
                finish_chunk(c - 1)

    nc.compile()
    return nc


def _get_program():
    if "nc" not in _CACHE:
        _CACHE["nc"] = _build_program()
    return _CACHE["nc"]


def make_core_inputs(q, k, v, key_padding_mask):
    """Shard full inputs into per-core input maps (host side).

    Layout work done here (part of sharding): head-slice, transpose Q/K to
    [d, s], cast to fp16, build ones-augmented V, mask -> additive bias.
    """
    q = np.asarray(q, dtype=np.float32)
    k = np.asarray(k, dtype=np.float32)
    v = np.asarray(v, dtype=np.float32)

    bias = np.where(key_padding_mask, 0.0, MASK_NEG).astype(np.float32)
    # bias[b, s] with s = 128*t + p  ->  [B, 128(p), KTILES(t)]
    bias = np.ascontiguousarray(bias.reshape(B, KTILES, 128).transpose(0, 2, 1))

    # [B, S, H, D] -> [B, H, D, S] fp16
    qt = np.ascontiguousarray(q.transpose(0, 2, 3, 1).astype(np.float16))
    kt = np.ascontiguousarray(k.transpose(0, 2, 3, 1).astype(np.float16))
    # V_aug: [B, H, 128(p), KTILES(t), VW] fp16 with ones in column D
    va = np.zeros((B, H, 128, KTILES, VW), dtype=np.float16)
    # v[b, s, h, d] with s = 128*t + p
    va[:, :, :, :, 0:D] = (
        v.reshape(B, KTILES, 128, H, D).transpose(0, 3, 2, 1, 4).astype(np.float16)
    )
    va[:, :, :, :, D] = 1.0

    in_maps = []
    for c in range(NCORES):
        sl = slice(c * H2, (c + 1) * H2)
        in_maps.append(
            {
                "qt": np.ascontiguousarray(qt[:, sl]),
                "kt": np.ascontiguousarray(kt[:, sl]),
                "va": np.ascontiguousarray(va[:, sl]),
                "bias": bias,
            }
        )
    return in_maps


def assemble_output(results):
    """Concatenate per-core [B, S, H2, D] outputs along the head axis."""
    return np.concatenate([results[c]["o"] for c in range(NCORES)], axis=2)


def kernel(q, k, v, key_padding_mask):
    from concourse.bass_utils import run_bass_kernel_spmd

    nc = _get_program()
    in_maps = make_core_inputs(q, k, v, key_padding_mask)
    res = run_bass_kernel_spmd(nc, in_maps, list(range(NCORES)))
    return assemble_output(res.results)
